# revision 18
# baseline (speedup 1.0000x reference)
"""FP4 (e2m1-packed) column-parallel Linear: y = x @ W^T + b on 8 NeuronCores.

Strategy (fp8 DoubleRow at the 157 TF/s fp8 peak)
-------------------------------------------------
- Tensor-parallel over out_features, x replicated.
- W dequantized host-side to fp8-e4m3 (exact for e2m1 values).  x quantized
  host-side to fp8-e4m3 (hi) plus an e4m3 residual (lo) for the 2048 k's with
  the largest quantization-error energy (global ranking; the contraction is
  permuted so those occupy k-planes 0..15).  Measured rel err on the fixed
  inputs: 1.84e-2 (< 2e-2 gate) vs 2.67e-2 for uncompensated fp8.
- Matmuls run in DoubleRow perf mode: stationary W-slot [128, 2, 128],
  moving x-slot [128, 2, 512] (free 512), out [128 o, 512 s] f32 in one PSUM
  bank.  Each o-tile accumulates 24 chunks = 16 hi (full K) + 8 lo
  (compensated half).  Warm HW rate: ~216 ns per DR matmul back-to-back.
- Work split: 86 o-tiles = 8*10 full + 6 shared remainder tiles split into 24
  quarter-jobs of 6 chunks, 3 per core: every core runs 10*24 + 3*6 = 258
  DR matmuls (the global optimum 2064/8).  Host sums the quarter-job f32
  partials (plus bias) while unsharding.

Block-rotation sharding (kills the 1.5x W duplication of the naive layout)
--------------------------------------------------------------------------
k-space is split into 16 k-blocks of 256 (2 k-planes each); comp blocks are
0..7 (top-energy k's).  Device chunk positions p=0..23 on core c carry:
    p in 0..7:    hi product of block (p+c)%8        -> W storage block p
    p in 8..15:   hi product of block 8+((p-8+c)%8)  -> W storage block p
    p in 16..23:  lo product of block (p-16+c)%8     -> W storage block p-16
The W storage index is core-INdependent, so the lo chunks alias the hi W
planes: wt is [128, 32, 128] per o-tile (32 planes, not 48).  xs stays
48 planes (hi planes 0..31 + lo planes 32..47), laid out per-core.
Remainder jobs: slot s=0..2 on core c covers tile t = s + 3*(c%2) with the 6
products at positions {2s, 2s+1, 8+2s, 8+2s+1, 16+2s, 16+2s+1}; over the 4
same-parity cores each (tile, product) is covered exactly once.  wr per slot
is [128, 8, 128] (4 W blocks; the lo products alias the first two).

Host layouts (kperm = global k permutation, plane_k = kperm.reshape(32,128)):
    xs  [128, 48, 512]      f8
    wt  [10, 128, 32, 128]  f8   (per full o-tile, storage-block order)
    wr  [3, 128, 8, 128]    f8   (per job slot)
    bias[128, 10] f32
Outputs: yt [1280, 512] bf16 (full tiles), yr [3, 128, 512] f32 (partials).
"""

import numpy as np
import ml_dtypes

try:
    import concourse.bass as bass
except ImportError:
    import sys

    sys.path.insert(0, "/opt/trn_rl_repo")
    import concourse.bass as bass

import concourse.mybir as mybir
import concourse.tile as tile
from concourse import bacc
from concourse.bass_utils import run_bass_kernel_spmd

B, S, IN, OUT = 4, 128, 4096, 11008
NC = 8
SEQ = B * S  # 512
KT = 32  # hi k-planes of 128
NHI, NLO = 16, 8  # hi chunks (full K), lo compensation chunks
NCH = NHI + NLO  # 24 chunks per o-tile
NXP = 2 * NCH  # 48 xs planes
NWP = 2 * NHI  # 32 wt storage planes per full tile
CK = NLO * 256  # 2048 compensated k's
FULL_TILES = 10
O_PER_CORE = FULL_TILES * 128  # 1280
R_BASE = NC * O_PER_CORE  # 10240
R_TILES = 6
JOBS = 3  # job slots per core (6 chunks each)

_E2M1_F32 = np.array(
    [0.0, 0.5, 1.0, 1.5, 2.0, 3.0, 4.0, 6.0,
     -0.0, -0.5, -1.0, -1.5, -2.0, -3.0, -4.0, -6.0],
    dtype=np.float32,
)
_LUT_FP8 = _E2M1_F32.astype(ml_dtypes.float8_e4m3).view(np.uint8)  # [16]

_COMPILED = {}


def _pos_prod(u):
    """Execution position u (0..23) -> ('hi', h) or ('lo', l).

    Order is hi,hi,lo repeating: lo chunks need no new W planes (they alias
    the hi storage), so interleaving them flattens the DMA demand curve from
    ~246 GB/s to a uniform ~197 GB/s during the chunk-outer phase.
    """
    g3, r = divmod(u, 3)
    return ("hi", 2 * g3 + r) if r < 2 else ("lo", g3)


def _wstore(u):
    """Execution position u -> wt storage block index (lo aliases hi)."""
    kind, idx = _pos_prod(u)
    return idx


def _build_nc():
    nc = bacc.Bacc(
        "TRN2", target_bir_lowering=False, debug=False, num_devices=NC
    )
    f8 = mybir.dt.float8e4
    bf16 = mybir.dt.bfloat16
    f32 = mybir.dt.float32
    DR = mybir.MatmulPerfMode.DoubleRow

    xs_d = nc.dram_tensor("xs", [128, NXP, SEQ], f8, kind="ExternalInput")
    wt_d = nc.dram_tensor("wt", [FULL_TILES, 128, NWP, 128], f8, kind="ExternalInput")
    wr_d = nc.dram_tensor("wr", [JOBS, 128, 8, 128], f8, kind="ExternalInput")
    b_d = nc.dram_tensor("bias", [128, FULL_TILES], f32, kind="ExternalInput")
    y_d = nc.dram_tensor("yt", [O_PER_CORE, SEQ], bf16, kind="ExternalOutput")
    yr_d = nc.dram_tensor("yr", [JOBS, 128, SEQ], f32, kind="ExternalOutput")

    from contextlib import ExitStack

    with tile.TileContext(nc) as tc, ExitStack() as ctx:
        xp = ctx.enter_context(tc.tile_pool(name="x", bufs=1))
        wp = ctx.enter_context(tc.tile_pool(name="w", bufs=1))
        pp = ctx.enter_context(tc.tile_pool(name="psum", bufs=1, space="PSUM"))
        op = ctx.enter_context(tc.tile_pool(name="out", bufs=8))
        bp = ctx.enter_context(tc.tile_pool(name="bias", bufs=1))

        dma_rr = [0]

        def in_dma(out_ap, in_ap):
            eng = nc.sync if dma_rr[0] % 2 == 0 else nc.gpsimd
            dma_rr[0] += 1
            eng.dma_start(out_ap, in_ap)

        # PE warmup (HAM clock-gate): split the memsets across gpsimd and
        # vector so the warmup matmuls start ~7.0us and bridge the HAM busy
        # window until the first real data lands (~9.5us).
        dj_l = xp.tile([128, 128], bf16, tag="dj_l", name="dj_l")
        dj_r = xp.tile([128, SEQ], bf16, tag="dj_r", name="dj_r")
        nc.gpsimd.memset(dj_l[:], 0.0)
        nc.vector.memset(dj_r[:], 0.0)
        ps_w = pp.tile([128, SEQ], f32, tag="ps0", name="ps_w")
        for _ in range(4):
            nc.tensor.matmul(
                ps_w[:, 0:320], lhsT=dj_l[:], rhs=dj_r[:, 0:320],
                start=True, stop=True,
            )

        xs_t = xp.tile([128, NXP, SEQ], f8, tag="xs", name="xs")
        wts = [
            wp.tile([128, NWP, 128], f8, tag=f"w{j}", name=f"w{j}")
            for j in range(FULL_TILES)
        ]
        wrs = [
            wp.tile([128, 8, 128], f8, tag=f"wr{s}", name=f"wr{s}")
            for s in range(JOBS)
        ]
        bt = bp.tile([128, FULL_TILES], f32)

        # DMA schedule.  Group A (tiles 0-5, chunk-outer) consumes execution
        # positions u at ~1.3us pace; xs planes are stored in consumption
        # order, W stage q (storage blocks 4q..4q+3) is first needed at
        # position 6q.  Transfers are emitted in deadline order; uniform
        # demand ~197 GB/s.  Group B tiles stream afterwards.
        NA = 7  # group A size

        def xfer_x(t):  # xs positions 4t..4t+3 (8 planes), 512KB
            in_dma(xs_t[:, 8 * t:8 * t + 8, :], xs_d[:, 8 * t:8 * t + 8, :])

        def xfer_w(j, q):  # storage blocks 4q..4q+3 of tile j, 131KB
            in_dma(wts[j][:, 8 * q:8 * q + 8, :], wt_d[j, :, 8 * q:8 * q + 8, :])

        # The DMA engines ramp slowly (~100 GB/s for the first ~2us), so the
        # first-needed data ships in small pieces, and everything is emitted
        # in consumption-deadline order: xs pair u is consumed at ~S+1.3u us,
        # W storage block s at the matching hi chunk's position.
        def xfer_x2(t):  # xs positions 2t..2t+1 (4 planes), 256KB
            in_dma(xs_t[:, 4 * t:4 * t + 4, :], xs_d[:, 4 * t:4 * t + 4, :])

        in_dma(xs_t[:, 0:2, :], xs_d[:, 0:2, :])  # position 0 (128KB)
        in_dma(xs_t[:, 2:4, :], xs_d[:, 2:4, :])  # position 1
        for j in range(0, 3):
            xfer_w(j, 0)
        xfer_x2(1)
        for j in range(3, NA):
            xfer_w(j, 0)
        xfer_x2(2)
        in_dma(bt[:], b_d[:])
        xfer_x2(3)
        for j in range(0, 3):
            xfer_w(j, 1)
        xfer_x2(4)
        for j in range(3, NA):
            xfer_w(j, 1)
        xfer_x2(5)
        xfer_x2(6)
        for j in range(0, 3):
            xfer_w(j, 2)
        xfer_x2(7)
        for j in range(3, NA):
            xfer_w(j, 2)
        xfer_x2(8)
        xfer_x2(9)
        for j in range(0, 3):
            xfer_w(j, 3)
        xfer_x2(10)
        for j in range(3, NA):
            xfer_w(j, 3)
        xfer_x2(11)
        for j in range(NA, FULL_TILES):  # group B tiles, whole (524KB each)
            in_dma(wts[j][:], wt_d[j])
        for s in range(JOBS):
            in_dma(wrs[s][:], wr_d[s])

        # Job slot s: 6 products at fixed positions; wr blocks 0..3 hold the
        # W for positions 2s, 2s+1, 8+2s, 8+2s+1; lo positions alias 0..1.
        def do_job(s):
            psj = pp.tile([128, SEQ], f32, tag=f"ps{2 + s}", name=f"psj{s}")
            # products: hi 2s, 2s+1 (u=3s, 3s+1), hi 8+2s, 8+2s+1
            # (u=12+3s, 13+3s), lo 2s, 2s+1 (u=6s+2, 6s+5)
            seq = [
                (3 * s, 0), (3 * s + 1, 1),
                (12 + 3 * s, 2), (13 + 3 * s, 3),
                (6 * s + 2, 0), (6 * s + 5, 1),
            ]
            for u, (p, b) in enumerate(seq):
                nc.tensor.matmul(
                    psj[:],
                    lhsT=wrs[s][:, 2 * b:2 * b + 2, :],
                    rhs=xs_t[:, 2 * p:2 * p + 2, :],
                    start=(u == 0),
                    stop=(u == 5),
                    perf_mode=DR,
                )
            orb = op.tile([128, SEQ], f32, tag="or", name=f"or{s}")
            h = SEQ // 2
            nc.scalar.copy(orb[:, 0:h], psj[:, 0:h])
            nc.vector.tensor_copy(orb[:, h:SEQ], psj[:, h:SEQ])
            nc.sync.dma_start(yr_d[s], orb[:])

        def drain_tile(j, ps):
            ob = op.tile([128, SEQ], bf16, tag="ob", name=f"ob{j}")
            h = SEQ // 2
            nc.scalar.activation(
                ob[:, 0:h], ps[:, 0:h],
                mybir.ActivationFunctionType.Identity,
                bias=bt[:, j:j + 1], scale=1.0,
            )
            nc.vector.tensor_scalar_add(ob[:, h:SEQ], ps[:, h:SEQ], bt[:, j:j + 1])
            oeng = nc.scalar if j < NA else nc.sync
            oeng.dma_start(y_d[j * 128:(j + 1) * 128, :], ob[:])

        # Group A: tiles 0-5 chunk-outer (PE never starves during initial fill)
        psA = [pp.tile([128, SEQ], f32, tag=f"ps{j}", name=f"ps{j}") for j in range(NA)]
        for c in range(NCH):
            hs = _wstore(c)
            for j in range(NA):
                nc.tensor.matmul(
                    psA[j][:],
                    lhsT=wts[j][:, 2 * hs:2 * hs + 2, :],
                    rhs=xs_t[:, 2 * c:2 * c + 2, :],
                    start=(c == 0),
                    stop=(c == NCH - 1),
                    perf_mode=DR,
                )
        for j in range(NA):
            drain_tile(j, psA[j])

        # Group B: tiles 7-9 tile-outer (all data resident), jobs mid-stream.
        # Tile 7 takes the never-used bank ps7 so its start doesn't wait on
        # the group A drains; tiles 8-9 reuse ps0-1 (drained earlier).
        for j in range(NA, FULL_TILES):
            ps = pp.tile(
                [128, SEQ], f32,
                tag="ps7" if j == NA else f"ps{j - NA - 1}", name=f"ps{j}",
            )
            for c in range(NCH):
                hs = _wstore(c)
                nc.tensor.matmul(
                    ps[:],
                    lhsT=wts[j][:, 2 * hs:2 * hs + 2, :],
                    rhs=xs_t[:, 2 * c:2 * c + 2, :],
                    start=(c == 0),
                    stop=(c == NCH - 1),
                    perf_mode=DR,
                )
            drain_tile(j, ps)
            if j == NA + 1:
                for s in range(JOBS):
                    do_job(s)

    nc.compile()
    return nc


def _prep_inputs(x, weight_packed, bias_packed):
    x = np.asarray(x)
    xf = x.reshape(SEQ, IN).astype(np.float32)
    wp_ = np.asarray(weight_packed).astype(np.uint8)  # [OUT, IN//2]
    bp_ = np.asarray(bias_packed).astype(np.uint8)  # [OUT//2]

    # fp4 codes -> fp8-e4m3 bytes (exact)
    w8 = np.empty((OUT, IN), dtype=np.uint8)
    w8[:, 0::2] = _LUT_FP8[wp_ & 15]
    w8[:, 1::2] = _LUT_FP8[wp_ >> 4]
    wcodes = np.empty((OUT, IN), np.uint8)
    wcodes[:, 0::2] = wp_ & 15
    wcodes[:, 1::2] = wp_ >> 4
    wf = _E2M1_F32[wcodes]  # [OUT, IN] f32

    bcodes = np.empty((OUT,), np.uint8)
    bcodes[0::2] = bp_ & 15
    bcodes[1::2] = bp_ >> 4
    bias = _E2M1_F32[bcodes]  # [OUT] f32
    _COMPILED["bias_vals"] = bias

    # x hi/lo e4m3 split (TRN fp8e4 == ml_dtypes.float8_e4m3 in range)
    x8 = xf.astype(ml_dtypes.float8_e4m3)  # [SEQ, IN]
    lo = xf - x8.astype(np.float32)
    lo8 = lo.astype(ml_dtypes.float8_e4m3)
    x8b = np.ascontiguousarray(x8.view(np.uint8).T)   # [IN, SEQ]
    lo8b = np.ascontiguousarray(lo8.view(np.uint8).T)

    # Global adaptive selection: top-CK k's by residual energy -> planes 0..15
    lo_en = (lo.astype(np.float64) ** 2).sum(0)
    w2 = (wf.astype(np.float64) ** 2).sum(0)
    kperm = np.argsort(-(lo_en * w2), kind="stable")
    plane_k = kperm.reshape(KT, 128)  # [32, 128] k index of hi plane, partition

    def block_planes(b):  # k-block b -> its two k-plane row-index arrays
        return plane_k[2 * b], plane_k[2 * b + 1]

    in_maps = []
    for c in range(NC):
        # xs: execution position u -> (source, k-block) under the per-core
        # shift; planes stored in execution (hi,hi,lo) order.
        xs = np.empty((NXP, 128, SEQ), np.uint8)
        for u in range(NCH):
            kind, idx = _pos_prod(u)
            if kind == "hi":
                if idx < 8:
                    b, src = (idx + c) % 8, x8b
                else:
                    b, src = 8 + ((idx - 8 + c) % 8), x8b
            else:
                b, src = (idx + c) % 8, lo8b
            r0, r1 = block_planes(b)
            xs[2 * u] = src[r0]
            xs[2 * u + 1] = src[r1]
        xs = np.ascontiguousarray(xs.transpose(1, 0, 2)).view(ml_dtypes.float8_e4m3)

        # wt: storage block s -> k-block (s+c)%8 or 8+((s-8+c)%8)
        wrows = np.empty((NWP, 128), np.int64)
        for s_ in range(NHI):
            b = (s_ + c) % 8 if s_ < 8 else 8 + ((s_ - 8 + c) % 8)
            r0, r1 = block_planes(b)
            wrows[2 * s_], wrows[2 * s_ + 1] = r0, r1
        cols_flat = wrows.reshape(-1)  # [32*128] k columns in storage order
        rows_o = slice(c * O_PER_CORE, (c + 1) * O_PER_CORE)
        wt = np.ascontiguousarray(
            w8[rows_o][:, cols_flat]
            .reshape(FULL_TILES, 128, NWP, 128)
            .transpose(0, 3, 2, 1)
        ).view(ml_dtypes.float8_e4m3)  # [10, 128, 32, 128]

        # wr: slot s covers remainder tile t = s + 3*(c%2); blocks
        # (2s+c)%8, (2s+1+c)%8, 8+(2s+c)%8, 8+(2s+1+c)%8.
        wr = np.empty((JOBS, 128, 8, 128), dtype=np.uint8)
        for s_ in range(JOBS):
            t = s_ + 3 * (c % 2)
            blocks = [
                (2 * s_ + c) % 8,
                (2 * s_ + 1 + c) % 8,
                8 + ((2 * s_ + c) % 8),
                8 + ((2 * s_ + 1 + c) % 8),
            ]
            rrows = np.empty((8, 128), np.int64)
            for i, b in enumerate(blocks):
                r0, r1 = block_planes(b)
                rrows[2 * i], rrows[2 * i + 1] = r0, r1
            cols = rrows.reshape(-1)
            blk = w8[R_BASE + t * 128:R_BASE + (t + 1) * 128][:, cols]
            wr[s_] = blk.reshape(128, 8, 128).transpose(2, 1, 0)
        wr = np.ascontiguousarray(wr).view(ml_dtypes.float8_e4m3)

        bt = np.ascontiguousarray(
            bias[rows_o].reshape(FULL_TILES, 128).T.astype(np.float32)
        )
        in_maps.append({"xs": xs, "wt": wt, "wr": wr, "bias": bt})
    return in_maps


def _run(in_maps, **kwargs):
    if "nc" not in _COMPILED:
        _COMPILED["nc"] = _build_nc()
    return run_bass_kernel_spmd(_COMPILED["nc"], in_maps, list(range(NC)), **kwargs)


def _assemble(res):
    y = np.empty((SEQ, OUT), dtype=ml_dtypes.bfloat16)
    racc = np.zeros((R_TILES, 128, SEQ), dtype=np.float32)
    for c in range(NC):
        yt = np.asarray(res.results[c]["yt"])  # [1280, SEQ] bf16
        y[:, c * O_PER_CORE:(c + 1) * O_PER_CORE] = yt.T
        yr = np.asarray(res.results[c]["yr"])  # [3, 128, SEQ] f32
        for s in range(JOBS):
            t = s + 3 * (c % 2)
            racc[t] += yr[s]
    rbias = _COMPILED["bias_vals"][R_BASE:]  # [768] f32
    for i in range(R_TILES):
        tile_f32 = racc[i] + rbias[i * 128:(i + 1) * 128][:, None]
        y[:, R_BASE + i * 128:R_BASE + (i + 1) * 128] = (
            tile_f32.T.astype(ml_dtypes.bfloat16)
        )
    return y.reshape(B, S, OUT)


def kernel(x, weight_packed, bias_packed, _bass_results=None):
    in_maps = _prep_inputs(x, weight_packed, bias_packed)
    res = _run(in_maps)
    if _bass_results is not None:
        _bass_results.append(res)
    return _assemble(res)


# revision 19
# speedup vs baseline: 1.0322x; 1.0322x over previous
"""FP4 (e2m1-packed) column-parallel Linear: y = x @ W^T + b on 8 NeuronCores.

Strategy (fp8 DoubleRow at the 157 TF/s fp8 peak)
-------------------------------------------------
- Tensor-parallel over out_features, x replicated.
- W dequantized host-side to fp8-e4m3 (exact for e2m1 values).  x quantized
  host-side to fp8-e4m3 (hi) plus an e4m3 residual (lo) for the 2048 k's with
  the largest quantization-error energy (global ranking; the contraction is
  permuted so those occupy k-planes 0..15).  Measured rel err on the fixed
  inputs: 1.84e-2 (< 2e-2 gate) vs 2.67e-2 for uncompensated fp8.
- Matmuls run in DoubleRow perf mode: stationary W-slot [128, 2, 128],
  moving x-slot [128, 2, 512] (free 512), out [128 o, 512 s] f32 in one PSUM
  bank.  Each o-tile accumulates 24 chunks = 16 hi (full K) + 8 lo
  (compensated half).  Warm HW rate: ~216 ns per DR matmul back-to-back.
- Work split: 86 o-tiles = 8*10 full + 6 shared remainder tiles split into 24
  quarter-jobs of 6 chunks, 3 per core: every core runs 10*24 + 3*6 = 258
  DR matmuls (the global optimum 2064/8).  Host sums the quarter-job f32
  partials (plus bias) while unsharding.

Block-rotation sharding (kills the 1.5x W duplication of the naive layout)
--------------------------------------------------------------------------
k-space is split into 16 k-blocks of 256 (2 k-planes each); comp blocks are
0..7 (top-energy k's).  Device chunk positions p=0..23 on core c carry:
    p in 0..7:    hi product of block (p+c)%8        -> W storage block p
    p in 8..15:   hi product of block 8+((p-8+c)%8)  -> W storage block p
    p in 16..23:  lo product of block (p-16+c)%8     -> W storage block p-16
The W storage index is core-INdependent, so the lo chunks alias the hi W
planes: wt is [128, 32, 128] per o-tile (32 planes, not 48).  xs stays
48 planes (hi planes 0..31 + lo planes 32..47), laid out per-core.
Remainder jobs: slot s=0..2 on core c covers tile t = s + 3*(c%2) with the 6
products at positions {2s, 2s+1, 8+2s, 8+2s+1, 16+2s, 16+2s+1}; over the 4
same-parity cores each (tile, product) is covered exactly once.  wr per slot
is [128, 8, 128] (4 W blocks; the lo products alias the first two).

Host layouts (kperm = global k permutation, plane_k = kperm.reshape(32,128)):
    xs  [128, 48, 512]      f8
    wt  [10, 128, 32, 128]  f8   (per full o-tile, storage-block order)
    wr  [3, 128, 8, 128]    f8   (per job slot)
    bias[128, 10] f32
Outputs: yt [1280, 512] bf16 (full tiles), yr [3, 128, 512] f32 (partials).
"""

import numpy as np
import ml_dtypes

try:
    import concourse.bass as bass
except ImportError:
    import sys

    sys.path.insert(0, "/opt/trn_rl_repo")
    import concourse.bass as bass

import concourse.mybir as mybir
import concourse.tile as tile
from concourse import bacc
from concourse.bass_utils import run_bass_kernel_spmd

B, S, IN, OUT = 4, 128, 4096, 11008
NC = 8
SEQ = B * S  # 512
KT = 32  # hi k-planes of 128
NHI, NLO = 16, 8  # hi chunks (full K), lo compensation chunks
NCH = NHI + NLO  # 24 chunks per o-tile
NXP = 2 * NCH  # 48 xs planes
NWP = 2 * NHI  # 32 wt storage planes per full tile
CK = NLO * 256  # 2048 compensated k's
FULL_TILES = 10
O_PER_CORE = FULL_TILES * 128  # 1280
R_BASE = NC * O_PER_CORE  # 10240
R_TILES = 6
JOBS = 3  # job slots per core (6 chunks each)

_E2M1_F32 = np.array(
    [0.0, 0.5, 1.0, 1.5, 2.0, 3.0, 4.0, 6.0,
     -0.0, -0.5, -1.0, -1.5, -2.0, -3.0, -4.0, -6.0],
    dtype=np.float32,
)
_LUT_FP8 = _E2M1_F32.astype(ml_dtypes.float8_e4m3).view(np.uint8)  # [16]

_COMPILED = {}


def _pos_prod(u):
    """Execution position u (0..23) -> ('hi', h) or ('lo', l).

    Order is hi,hi,lo repeating: lo chunks need no new W planes (they alias
    the hi storage), so interleaving them flattens the DMA demand curve from
    ~246 GB/s to a uniform ~197 GB/s during the chunk-outer phase.
    """
    g3, r = divmod(u, 3)
    return ("hi", 2 * g3 + r) if r < 2 else ("lo", g3)


def _wstore(u):
    """Execution position u -> wt storage block index (lo aliases hi)."""
    kind, idx = _pos_prod(u)
    return idx


def _build_nc():
    nc = bacc.Bacc(
        "TRN2", target_bir_lowering=False, debug=False, num_devices=NC
    )
    f8 = mybir.dt.float8e4
    bf16 = mybir.dt.bfloat16
    f32 = mybir.dt.float32
    DR = mybir.MatmulPerfMode.DoubleRow

    xs_d = nc.dram_tensor("xs", [128, NXP, SEQ], f8, kind="ExternalInput")
    wt_d = nc.dram_tensor("wt", [FULL_TILES, 128, NWP, 128], f8, kind="ExternalInput")
    wr_d = nc.dram_tensor("wr", [JOBS, 128, 8, 128], f8, kind="ExternalInput")
    b_d = nc.dram_tensor("bias", [128, FULL_TILES], f32, kind="ExternalInput")
    y_d = nc.dram_tensor("yt", [O_PER_CORE, SEQ], bf16, kind="ExternalOutput")
    yr_d = nc.dram_tensor("yr", [JOBS, 128, SEQ], f32, kind="ExternalOutput")

    from contextlib import ExitStack

    with tile.TileContext(nc) as tc, ExitStack() as ctx:
        xp = ctx.enter_context(tc.tile_pool(name="x", bufs=1))
        wp = ctx.enter_context(tc.tile_pool(name="w", bufs=1))
        pp = ctx.enter_context(tc.tile_pool(name="psum", bufs=1, space="PSUM"))
        op = ctx.enter_context(tc.tile_pool(name="out", bufs=8))
        bp = ctx.enter_context(tc.tile_pool(name="bias", bufs=1))

        dma_rr = [0]

        def in_dma(out_ap, in_ap):
            eng = nc.sync if dma_rr[0] % 2 == 0 else nc.gpsimd
            dma_rr[0] += 1
            eng.dma_start(out_ap, in_ap)

        # PE warmup (HAM clock-gate): split the memsets across gpsimd and
        # vector so the warmup matmuls start ~7.0us and bridge the HAM busy
        # window until the first real data lands (~9.5us).
        dj_l = xp.tile([128, 128], bf16, tag="dj_l", name="dj_l")
        dj_r = xp.tile([128, SEQ], bf16, tag="dj_r", name="dj_r")
        nc.gpsimd.memset(dj_l[:], 0.0)
        nc.vector.memset(dj_r[:], 0.0)
        ps_w = pp.tile([128, SEQ], f32, tag="ps7", name="ps_w")
        for _ in range(4):
            nc.tensor.matmul(
                ps_w[:, 0:320], lhsT=dj_l[:], rhs=dj_r[:, 0:320],
                start=True, stop=True,
            )

        xs_t = xp.tile([128, NXP, SEQ], f8, tag="xs", name="xs")
        wts = [
            wp.tile([128, NWP, 128], f8, tag=f"w{j}", name=f"w{j}")
            for j in range(FULL_TILES)
        ]
        wrs = [
            wp.tile([128, 8, 128], f8, tag=f"wr{s}", name=f"wr{s}")
            for s in range(JOBS)
        ]
        bt = bp.tile([128, FULL_TILES], f32)

        # DMA schedule.  Group A (tiles 0-5, chunk-outer) consumes execution
        # positions u at ~1.3us pace; xs planes are stored in consumption
        # order, W stage q (storage blocks 4q..4q+3) is first needed at
        # position 6q.  Transfers are emitted in deadline order; uniform
        # demand ~197 GB/s.  Group B tiles stream afterwards.
        NA = 7  # group A size

        def xfer_x(t):  # xs positions 4t..4t+3 (8 planes), 512KB
            in_dma(xs_t[:, 8 * t:8 * t + 8, :], xs_d[:, 8 * t:8 * t + 8, :])

        def xfer_w(j, q):  # storage blocks 4q..4q+3 of tile j, 131KB
            in_dma(wts[j][:, 8 * q:8 * q + 8, :], wt_d[j, :, 8 * q:8 * q + 8, :])

        # The DMA engines ramp slowly (~100 GB/s for the first ~2us), so the
        # first-needed data ships in small pieces, and everything is emitted
        # in consumption-deadline order: xs pair u is consumed at ~S+1.3u us,
        # W storage block s at the matching hi chunk's position.
        def xfer_x2(t):  # xs positions 2t..2t+1 (4 planes), 256KB
            in_dma(xs_t[:, 4 * t:4 * t + 4, :], xs_d[:, 4 * t:4 * t + 4, :])

        in_dma(xs_t[:, 0:2, :], xs_d[:, 0:2, :])  # position 0 -> sync
        xfer_w(0, 0)                               # tile 0 W -> gpsimd
        in_dma(xs_t[:, 2:4, :], xs_d[:, 2:4, :])  # position 1 -> sync
        for j in range(1, NA):
            xfer_w(j, 0)
        xfer_x2(1)
        in_dma(bt[:], b_d[:])
        xfer_x2(2)
        xfer_x2(3)
        for j in range(0, 3):
            xfer_w(j, 1)
        xfer_x2(4)
        for j in range(3, NA):
            xfer_w(j, 1)
        xfer_x2(5)
        xfer_x2(6)
        for j in range(0, 3):
            xfer_w(j, 2)
        xfer_x2(7)
        for j in range(3, NA):
            xfer_w(j, 2)
        xfer_x2(8)
        xfer_x2(9)
        for j in range(0, 3):
            xfer_w(j, 3)
        xfer_x2(10)
        for j in range(3, NA):
            xfer_w(j, 3)
        xfer_x2(11)
        for j in range(NA, FULL_TILES):  # group B tiles, whole (524KB each)
            in_dma(wts[j][:], wt_d[j])
        for s in range(JOBS):
            in_dma(wrs[s][:], wr_d[s])

        # Job slot s: 6 products at fixed positions; wr blocks 0..3 hold the
        # W for positions 2s, 2s+1, 8+2s, 8+2s+1; lo positions alias 0..1.
        def do_job(s):
            psj = pp.tile([128, SEQ], f32, tag=f"ps{2 + s}", name=f"psj{s}")
            # products: hi 2s, 2s+1 (u=3s, 3s+1), hi 8+2s, 8+2s+1
            # (u=12+3s, 13+3s), lo 2s, 2s+1 (u=6s+2, 6s+5)
            seq = [
                (3 * s, 0), (3 * s + 1, 1),
                (12 + 3 * s, 2), (13 + 3 * s, 3),
                (6 * s + 2, 0), (6 * s + 5, 1),
            ]
            for u, (p, b) in enumerate(seq):
                nc.tensor.matmul(
                    psj[:],
                    lhsT=wrs[s][:, 2 * b:2 * b + 2, :],
                    rhs=xs_t[:, 2 * p:2 * p + 2, :],
                    start=(u == 0),
                    stop=(u == 5),
                    perf_mode=DR,
                )
            orb = op.tile([128, SEQ], f32, tag="or", name=f"or{s}")
            h = SEQ // 2
            nc.scalar.copy(orb[:, 0:h], psj[:, 0:h])
            nc.vector.tensor_copy(orb[:, h:SEQ], psj[:, h:SEQ])
            nc.sync.dma_start(yr_d[s], orb[:])

        def drain_tile(j, ps):
            ob = op.tile([128, SEQ], bf16, tag="ob", name=f"ob{j}")
            h = SEQ // 2
            nc.scalar.activation(
                ob[:, 0:h], ps[:, 0:h],
                mybir.ActivationFunctionType.Identity,
                bias=bt[:, j:j + 1], scale=1.0,
            )
            nc.vector.tensor_scalar_add(ob[:, h:SEQ], ps[:, h:SEQ], bt[:, j:j + 1])
            oeng = nc.scalar if j < NA else nc.sync
            oeng.dma_start(y_d[j * 128:(j + 1) * 128, :], ob[:])

        # Group A: tiles 0-5 chunk-outer (PE never starves during initial fill)
        psA = [pp.tile([128, SEQ], f32, tag=f"ps{j}", name=f"ps{j}") for j in range(NA)]
        FILLERS = {0: 3, 1: 3, 2: 2, 3: 2}  # bridge DMA-ramp stalls (HAM)
        for c in range(NCH):
            hs = _wstore(c)
            for j in range(NA):
                nc.tensor.matmul(
                    psA[j][:],
                    lhsT=wts[j][:, 2 * hs:2 * hs + 2, :],
                    rhs=xs_t[:, 2 * c:2 * c + 2, :],
                    start=(c == 0),
                    stop=(c == NCH - 1),
                    perf_mode=DR,
                )
            for _ in range(FILLERS.get(c, 0)):
                nc.tensor.matmul(
                    ps_w[:, 0:320], lhsT=dj_l[:], rhs=dj_r[:, 0:320],
                    start=True, stop=True,
                )
        for j in range(NA):
            drain_tile(j, psA[j])

        # Group B: tiles 7-9 tile-outer (all data resident), jobs mid-stream.
        # Tile 7 takes the never-used bank ps7 so its start doesn't wait on
        # the group A drains; tiles 8-9 reuse ps0-1 (drained earlier).
        for j in range(NA, FULL_TILES):
            ps = pp.tile(
                [128, SEQ], f32,
                tag="ps7" if j == NA else f"ps{j - NA - 1}", name=f"ps{j}",
            )
            for c in range(NCH):
                hs = _wstore(c)
                nc.tensor.matmul(
                    ps[:],
                    lhsT=wts[j][:, 2 * hs:2 * hs + 2, :],
                    rhs=xs_t[:, 2 * c:2 * c + 2, :],
                    start=(c == 0),
                    stop=(c == NCH - 1),
                    perf_mode=DR,
                )
            drain_tile(j, ps)
            if j == NA + 1:
                for s in range(JOBS):
                    do_job(s)

    nc.compile()
    return nc


def _prep_inputs(x, weight_packed, bias_packed):
    x = np.asarray(x)
    xf = x.reshape(SEQ, IN).astype(np.float32)
    wp_ = np.asarray(weight_packed).astype(np.uint8)  # [OUT, IN//2]
    bp_ = np.asarray(bias_packed).astype(np.uint8)  # [OUT//2]

    # fp4 codes -> fp8-e4m3 bytes (exact)
    w8 = np.empty((OUT, IN), dtype=np.uint8)
    w8[:, 0::2] = _LUT_FP8[wp_ & 15]
    w8[:, 1::2] = _LUT_FP8[wp_ >> 4]
    wcodes = np.empty((OUT, IN), np.uint8)
    wcodes[:, 0::2] = wp_ & 15
    wcodes[:, 1::2] = wp_ >> 4
    wf = _E2M1_F32[wcodes]  # [OUT, IN] f32

    bcodes = np.empty((OUT,), np.uint8)
    bcodes[0::2] = bp_ & 15
    bcodes[1::2] = bp_ >> 4
    bias = _E2M1_F32[bcodes]  # [OUT] f32
    _COMPILED["bias_vals"] = bias

    # x hi/lo e4m3 split (TRN fp8e4 == ml_dtypes.float8_e4m3 in range)
    x8 = xf.astype(ml_dtypes.float8_e4m3)  # [SEQ, IN]
    lo = xf - x8.astype(np.float32)
    lo8 = lo.astype(ml_dtypes.float8_e4m3)
    x8b = np.ascontiguousarray(x8.view(np.uint8).T)   # [IN, SEQ]
    lo8b = np.ascontiguousarray(lo8.view(np.uint8).T)

    # Global adaptive selection: top-CK k's by residual energy -> planes 0..15
    lo_en = (lo.astype(np.float64) ** 2).sum(0)
    w2 = (wf.astype(np.float64) ** 2).sum(0)
    kperm = np.argsort(-(lo_en * w2), kind="stable")
    plane_k = kperm.reshape(KT, 128)  # [32, 128] k index of hi plane, partition

    def block_planes(b):  # k-block b -> its two k-plane row-index arrays
        return plane_k[2 * b], plane_k[2 * b + 1]

    in_maps = []
    for c in range(NC):
        # xs: execution position u -> (source, k-block) under the per-core
        # shift; planes stored in execution (hi,hi,lo) order.
        xs = np.empty((NXP, 128, SEQ), np.uint8)
        for u in range(NCH):
            kind, idx = _pos_prod(u)
            if kind == "hi":
                if idx < 8:
                    b, src = (idx + c) % 8, x8b
                else:
                    b, src = 8 + ((idx - 8 + c) % 8), x8b
            else:
                b, src = (idx + c) % 8, lo8b
            r0, r1 = block_planes(b)
            xs[2 * u] = src[r0]
            xs[2 * u + 1] = src[r1]
        xs = np.ascontiguousarray(xs.transpose(1, 0, 2)).view(ml_dtypes.float8_e4m3)

        # wt: storage block s -> k-block (s+c)%8 or 8+((s-8+c)%8)
        wrows = np.empty((NWP, 128), np.int64)
        for s_ in range(NHI):
            b = (s_ + c) % 8 if s_ < 8 else 8 + ((s_ - 8 + c) % 8)
            r0, r1 = block_planes(b)
            wrows[2 * s_], wrows[2 * s_ + 1] = r0, r1
        cols_flat = wrows.reshape(-1)  # [32*128] k columns in storage order
        rows_o = slice(c * O_PER_CORE, (c + 1) * O_PER_CORE)
        wt = np.ascontiguousarray(
            w8[rows_o][:, cols_flat]
            .reshape(FULL_TILES, 128, NWP, 128)
            .transpose(0, 3, 2, 1)
        ).view(ml_dtypes.float8_e4m3)  # [10, 128, 32, 128]

        # wr: slot s covers remainder tile t = s + 3*(c%2); blocks
        # (2s+c)%8, (2s+1+c)%8, 8+(2s+c)%8, 8+(2s+1+c)%8.
        wr = np.empty((JOBS, 128, 8, 128), dtype=np.uint8)
        for s_ in range(JOBS):
            t = s_ + 3 * (c % 2)
            blocks = [
                (2 * s_ + c) % 8,
                (2 * s_ + 1 + c) % 8,
                8 + ((2 * s_ + c) % 8),
                8 + ((2 * s_ + 1 + c) % 8),
            ]
            rrows = np.empty((8, 128), np.int64)
            for i, b in enumerate(blocks):
                r0, r1 = block_planes(b)
                rrows[2 * i], rrows[2 * i + 1] = r0, r1
            cols = rrows.reshape(-1)
            blk = w8[R_BASE + t * 128:R_BASE + (t + 1) * 128][:, cols]
            wr[s_] = blk.reshape(128, 8, 128).transpose(2, 1, 0)
        wr = np.ascontiguousarray(wr).view(ml_dtypes.float8_e4m3)

        bt = np.ascontiguousarray(
            bias[rows_o].reshape(FULL_TILES, 128).T.astype(np.float32)
        )
        in_maps.append({"xs": xs, "wt": wt, "wr": wr, "bias": bt})
    return in_maps


def _run(in_maps, **kwargs):
    if "nc" not in _COMPILED:
        _COMPILED["nc"] = _build_nc()
    return run_bass_kernel_spmd(_COMPILED["nc"], in_maps, list(range(NC)), **kwargs)


def _assemble(res):
    y = np.empty((SEQ, OUT), dtype=ml_dtypes.bfloat16)
    racc = np.zeros((R_TILES, 128, SEQ), dtype=np.float32)
    for c in range(NC):
        yt = np.asarray(res.results[c]["yt"])  # [1280, SEQ] bf16
        y[:, c * O_PER_CORE:(c + 1) * O_PER_CORE] = yt.T
        yr = np.asarray(res.results[c]["yr"])  # [3, 128, SEQ] f32
        for s in range(JOBS):
            t = s + 3 * (c % 2)
            racc[t] += yr[s]
    rbias = _COMPILED["bias_vals"][R_BASE:]  # [768] f32
    for i in range(R_TILES):
        tile_f32 = racc[i] + rbias[i * 128:(i + 1) * 128][:, None]
        y[:, R_BASE + i * 128:R_BASE + (i + 1) * 128] = (
            tile_f32.T.astype(ml_dtypes.bfloat16)
        )
    return y.reshape(B, S, OUT)


def kernel(x, weight_packed, bias_packed, _bass_results=None):
    in_maps = _prep_inputs(x, weight_packed, bias_packed)
    res = _run(in_maps)
    if _bass_results is not None:
        _bass_results.append(res)
    return _assemble(res)
